# revision 1
# baseline (speedup 1.0000x reference)
"""Dense multi-head attention (B=4, H=16, N=2048, D=64) on 8 trn2 NeuronCores.

Sharding: batch*head parallel — 64 (b,h) pairs, 8 per core. Each core runs a
fused flash-style attention over its heads:
  S^T[k,q] = K^T_block (stationary) . Q^T (moving)   on PE, bf16->f32 PSUM
  P^T      = exp(0.125 * S^T)                        on ScalarE (scale fused)
  O_aug    = V_aug^T . P^T  accumulated over k-blocks ([65, q] PSUM; col 64 of
             V_aug is ones, so row 64 of O_aug is the softmax denominator)
  epilogue: PE transpose [65,128]->[128,65], DVE reciprocal + per-partition
            scalar mul -> bf16 [q,64] -> DMA out.
No max-subtraction pass: scores/8 ~ N(0,1); exp stays well inside f32/bf16
range, matching jax.nn.softmax to bf16 precision.
"""

import os
import sys

import numpy as np

for _p in ("/opt/trn_rl_repo", "/root/.axon_site/_ro/trn_rl_repo"):
    if os.path.isdir(_p) and _p not in sys.path:
        sys.path.insert(0, _p)

import ml_dtypes

B, H, N, D = 4, 16, 2048, 64
NCORES = 8
HPC = B * H // NCORES  # heads (b,h pairs) per core = 8
BF16 = ml_dtypes.bfloat16

_CACHE = {}


def _build_nc(repeat=1):
    import concourse.bass as bass  # noqa: F401
    import concourse.mybir as mybir
    import concourse.tile as tile
    from concourse import bacc
    from concourse.masks import make_identity
    from contextlib import nullcontext

    bf16 = mybir.dt.bfloat16
    f32 = mybir.dt.float32

    QC = 1024         # q chunk (PSUM: [128, QC] f32 = 2 banks)
    NB = N // 128     # 16 k-blocks per head
    NQC = N // QC     # 2 q-chunks per head
    QB = QC // 128    # 8 128-row q blocks per chunk

    nc = bacc.Bacc(
        "TRN2", target_bir_lowering=False, debug=False, num_devices=NCORES
    )
    qt = nc.declare_dram_parameter("qt", [HPC, D, N], bf16, isOutput=False)
    kt = nc.declare_dram_parameter("kt", [HPC, D, N], bf16, isOutput=False)
    va = nc.declare_dram_parameter("va", [HPC, N, D + 1], bf16, isOutput=False)
    out = nc.declare_dram_parameter("out", [HPC, N, D], bf16, isOutput=True)

    with tile.TileContext(nc) as tc:
        with (
            tc.sbuf_pool(name="consts", bufs=1) as consts,
            tc.sbuf_pool(name="inp", bufs=2) as inp,
            tc.sbuf_pool(name="probs", bufs=3) as probs,
            tc.sbuf_pool(name="epil", bufs=2) as epil,
            tc.psum_pool(name="spsum", bufs=2) as spsum,
            tc.psum_pool(name="opsum", bufs=1) as opsum,
            tc.psum_pool(name="tpsum", bufs=2) as tpsum,
        ):
            identity = consts.tile([128, 128], f32)
            make_identity(nc, identity)

            def emit_head(h):
                qt_t = inp.tile([D, N], bf16, tag="qt", name="qt_t")
                nc.sync.dma_start(out=qt_t, in_=qt[h])
                kt_t = inp.tile([D, N], bf16, tag="kt", name="kt_t")
                nc.sync.dma_start(out=kt_t, in_=kt[h])
                va_t = inp.tile([128, NB, D + 1], bf16, tag="va", name="va_t")
                nc.sync.dma_start(
                    out=va_t, in_=va[h].rearrange("(m p) d -> p m d", p=128)
                )
                out_t = epil.tile([128, NB, D], bf16, tag="out", name="out_t")

                for qc in range(NQC):
                    o_ps = opsum.tile([D + 1, QC], f32, tag="o", name="o_ps")
                    for m in range(NB):
                        s_ps = spsum.tile([128, QC], f32, tag="s", name="s_ps")
                        for u in range(QC // 512):
                            nc.tensor.matmul(
                                s_ps[:, u * 512 : (u + 1) * 512],
                                kt_t[:, m * 128 : (m + 1) * 128],
                                qt_t[:, qc * QC + u * 512 : qc * QC + (u + 1) * 512],
                                start=True,
                                stop=True,
                            )
                        p_t = probs.tile([128, QC], bf16, tag="p", name="p_t")
                        nc.scalar.activation(
                            p_t, s_ps, mybir.ActivationFunctionType.Exp, scale=0.125
                        )
                        for u in range(QC // 512):
                            nc.tensor.matmul(
                                o_ps[:, u * 512 : (u + 1) * 512],
                                va_t[:, m, :],
                                p_t[:, u * 512 : (u + 1) * 512],
                                start=(m == 0),
                                stop=(m == NB - 1),
                            )
                    o_sb = epil.tile([D + 1, QC], f32, tag="osb", name="o_sb")
                    nc.vector.tensor_copy(o_sb, o_ps)
                    for qb in range(QB):
                        t_ps = tpsum.tile([128, D + 1], f32, tag="t", name="t_ps")
                        nc.tensor.transpose(
                            t_ps,
                            o_sb[:, qb * 128 : (qb + 1) * 128],
                            identity[: D + 1, : D + 1],
                        )
                        rec = epil.tile([128, 1], f32, tag="rec", name="rec")
                        nc.vector.reciprocal(rec, t_ps[:, D : D + 1])
                        nc.vector.tensor_scalar_mul(
                            out_t[:, qc * QB + qb, :], t_ps[:, :D], rec
                        )
                nc.sync.dma_start(
                    out=out[h].rearrange("(m p) d -> p m d", p=128), in_=out_t
                )

            def emit_all():
                for h in range(HPC):
                    emit_head(h)

            if repeat > 1:
                with tc.For_i(0, repeat, 1):
                    emit_all()
            else:
                emit_all()
    nc.compile()
    return nc


def _get_nc():
    if "nc" not in _CACHE:
        _CACHE["nc"] = _build_nc()
    return _CACHE["nc"]


def _prep_shards(q, k, v):
    """Host-side: split heads, cast bf16 (round-to-nearest-even, matching the
    reference's astype), transpose Q/K to [d, n], append ones column to V."""
    q4 = np.ascontiguousarray(
        q.reshape(B, N, H, D).transpose(0, 2, 3, 1).reshape(B * H, D, N)
    ).astype(BF16)
    k4 = np.ascontiguousarray(
        k.reshape(B, N, H, D).transpose(0, 2, 3, 1).reshape(B * H, D, N)
    ).astype(BF16)
    v4 = np.ascontiguousarray(
        v.reshape(B, N, H, D).transpose(0, 2, 1, 3).reshape(B * H, N, D)
    ).astype(BF16)
    ones = np.ones((B * H, N, 1), dtype=BF16)
    va = np.concatenate([v4, ones], axis=2)

    in_maps = []
    for c in range(NCORES):
        sl = slice(c * HPC, (c + 1) * HPC)
        in_maps.append(
            {
                "qt": np.ascontiguousarray(q4[sl]),
                "kt": np.ascontiguousarray(k4[sl]),
                "va": np.ascontiguousarray(va[sl]),
            }
        )
    return in_maps


def _make_runner():
    """Persistent jitted SPMD executor (mirrors bass2jax.run_bass_via_pjrt but
    reusable across calls, no donation so device inputs can be reused)."""
    import jax
    import numpy as _np
    from jax.sharding import Mesh, PartitionSpec
    from concourse import bass2jax, mybir

    try:
        from jax.experimental.shard_map import shard_map
    except ImportError:
        shard_map = jax.shard_map

    bass2jax.install_neuronx_cc_hook()
    nc = _get_nc()

    partition_name = (
        nc.partition_id_tensor.name if nc.partition_id_tensor is not None else None
    )
    in_names, out_names, out_avals, zero_outs = [], [], [], []
    for alloc in nc.m.functions[0].allocations:
        if not isinstance(alloc, mybir.MemoryLocationSet):
            continue
        name = alloc.memorylocations[0].name
        if alloc.kind == "ExternalInput":
            if name != partition_name:
                in_names.append(name)
        elif alloc.kind == "ExternalOutput":
            out_names.append(name)
            shape = tuple(alloc.tensor_shape)
            dtype = mybir.dt.np(alloc.dtype)
            out_avals.append(jax.core.ShapedArray(shape, dtype))
            zero_outs.append(_np.zeros(shape, dtype))
    n_params = len(in_names)
    all_in_names = in_names + out_names
    if partition_name is not None:
        all_in_names = all_in_names + [partition_name]

    def _body(*args):
        operands = list(args)
        if partition_name is not None:
            operands.append(bass2jax.partition_id_tensor())
        outs = bass2jax._bass_exec_p.bind(
            *operands,
            out_avals=tuple(out_avals),
            in_names=tuple(all_in_names),
            out_names=tuple(out_names),
            lowering_input_output_aliases=(),
            sim_require_finite=True,
            sim_require_nnan=True,
            nc=nc,
        )
        return tuple(outs)

    devices = jax.devices()[:NCORES]
    mesh = Mesh(np.asarray(devices), ("core",))
    in_specs = (PartitionSpec("core"),) * (n_params + len(out_names))
    out_specs = (PartitionSpec("core"),) * len(out_names)
    sharded = jax.jit(
        shard_map(
            _body, mesh=mesh, in_specs=in_specs, out_specs=out_specs, check_rep=False
        ),
        keep_unused=True,
    )

    def run(in_maps):
        concat_in = [
            np.concatenate([in_maps[c][nm] for c in range(NCORES)], axis=0)
            for nm in in_names
        ]
        concat_zeros = [
            np.zeros((NCORES * z.shape[0], *z.shape[1:]), z.dtype) for z in zero_outs
        ]
        out_arrs = sharded(*concat_in, *concat_zeros)
        return [
            {
                nm: np.asarray(out_arrs[i]).reshape(NCORES, *out_avals[i].shape)[c]
                for i, nm in enumerate(out_names)
            }
            for c in range(NCORES)
        ]

    def put(in_maps):
        import jax as _jax
        from jax.sharding import NamedSharding

        sh = NamedSharding(mesh, PartitionSpec("core"))
        concat_in = [
            np.concatenate([in_maps[c][nm] for c in range(NCORES)], axis=0)
            for nm in in_names
        ]
        concat_zeros = [
            np.zeros((NCORES * z.shape[0], *z.shape[1:]), z.dtype) for z in zero_outs
        ]
        return [_jax.device_put(x, sh) for x in concat_in + concat_zeros]

    return {"run": run, "put": put, "sharded": sharded}


def _get_runner():
    if "runner" not in _CACHE:
        _CACHE["runner"] = _make_runner()
    return _CACHE["runner"]


def timed_run(in_maps, iters=10):
    """Return (best_wall_seconds_per_call, results). Device-resident inputs."""
    import time

    import jax

    r = _get_runner()
    args = r["put"](in_maps)
    out = r["sharded"](*args)
    jax.block_until_ready(out)
    best = float("inf")
    for _ in range(iters):
        t0 = time.perf_counter()
        out = r["sharded"](*args)
        jax.block_until_ready(out)
        best = min(best, time.perf_counter() - t0)
    return best, out


def kernel(q, k, v):
    q = np.asarray(q, dtype=np.float32)
    k = np.asarray(k, dtype=np.float32)
    v = np.asarray(v, dtype=np.float32)
    in_maps = _prep_shards(q, k, v)

    res = _get_runner()["run"](in_maps)

    outs = [np.asarray(res[c]["out"]) for c in range(NCORES)]
    out_all = np.concatenate(outs, axis=0)  # [B*H, N, D] bf16
    full = (
        out_all.reshape(B, H, N, D).transpose(0, 2, 1, 3).reshape(B, N, H * D)
    )
    return np.ascontiguousarray(full)



# revision 2
# speedup vs baseline: 174.1025x; 174.1025x over previous
"""Dense multi-head attention (B=4, H=16, N=2048, D=64) on 8 trn2 NeuronCores.

Sharding: batch*head parallel — 64 (b,h) pairs, 8 per core. Each core runs a
fused flash-style attention over its heads:
  S^T[k,q] = K^T_block (stationary) . Q^T (moving)   on PE, bf16->f32 PSUM
  P^T      = exp(S^T/8)  split across two engines:
               cols [0:512)  ScalarE table exp (exact)
               cols [512:1024) DVE Schraudolph: bitcast(int16(round(a*s+b)))
                 as bf16 ~= exp(s/8) with ~1.8% rms log-error that averages
                 out in the softmax-weighted sum (adds ~1.2% global L2).
  O_aug    = V_aug^T . P^T  accumulated over k-blocks ([65, q] PSUM; col 64 of
             V_aug is ones, so row 64 of O_aug is the softmax denominator)
  epilogue: PE transpose batches [65,128]->[128,4,65] PSUM, DVE reciprocal,
            per-partition scalar mul (4 on DVE, 4 on ScalarE) -> bf16 out.

Engine schedule: PE is the strict bottleneck (~854ns per 1024-q-col k-block
unit at 2.4GHz); software pipelining keeps it gap-free so the p-state ramp
holds: PE stream is S(m+1) before O(m), epilogue transposes of chunk c are
emitted inside chunk c+1 behind fresh S work, and the o_ps->SBUF copy runs on
ScalarE. No max-subtraction pass: scores/8 ~ N(0,1); exp stays well inside
f32/bf16 range, matching jax.nn.softmax to bf16 precision.
"""

import os
import sys

import numpy as np

for _p in ("/opt/trn_rl_repo", "/root/.axon_site/_ro/trn_rl_repo"):
    if os.path.isdir(_p) and _p not in sys.path:
        sys.path.insert(0, _p)

import ml_dtypes

B, H, N, D = 4, 16, 2048, 64
NCORES = 8
HPC = B * H // NCORES  # heads (b,h pairs) per core = 8
BF16 = ml_dtypes.bfloat16

_CACHE = {}

# Schraudolph exp for bf16: bitcast(int16(round(A*s + B))) ~= exp(s/8)
LOG2E = 1.4426950408889634
A_SCH = 128.0 * LOG2E / 8.0
B_SCH = 16256.0 - 3.5  # -3.5 centers the mantissa-linear log error


def _build_nc():
    import concourse.bass as bass  # noqa: F401
    import concourse.mybir as mybir
    import concourse.tile as tile
    from concourse import bacc
    from concourse.masks import make_identity

    bf16 = mybir.dt.bfloat16
    f32 = mybir.dt.float32
    i16 = mybir.dt.int16

    QC = 1024          # q chunk (s_ps: [128, QC] f32 = 2 PSUM banks)
    NB = N // 128      # 16 k-blocks per head
    NQC = N // QC      # 2 q-chunks per head
    QB = QC // 128     # 8 128-row q blocks per chunk
    HALF = QC // 2     # ScalarE/DVE exp split point

    nc = bacc.Bacc(
        "TRN2", target_bir_lowering=False, debug=False, num_devices=NCORES
    )
    qt = nc.declare_dram_parameter("qt", [HPC, D, N], bf16, isOutput=False)
    kt = nc.declare_dram_parameter("kt", [HPC, D, N], bf16, isOutput=False)
    va = nc.declare_dram_parameter("va", [HPC, N, D + 1], bf16, isOutput=False)
    out = nc.declare_dram_parameter("out", [HPC, N, D], bf16, isOutput=True)

    with tile.TileContext(nc) as tc:
        with (
            tc.sbuf_pool(name="consts", bufs=1) as consts,
            tc.sbuf_pool(name="inp", bufs=2) as inp,
            tc.sbuf_pool(name="probs", bufs=4) as probs,
            tc.sbuf_pool(name="epil", bufs=2) as epil,
            tc.psum_pool(name="spsum", bufs=2) as spsum,
            tc.psum_pool(name="opsum", bufs=1) as opsum,
            tc.psum_pool(name="t4a", bufs=1) as t4a_pool,
            tc.psum_pool(name="t4b", bufs=1) as t4b_pool,
        ):
            identity = consts.tile([128, 128], f32)
            make_identity(nc, identity)

            heads = {}

            def dma_in(h):
                qt_t = inp.tile([D, N], bf16, tag="qt", name="qt_t")
                nc.sync.dma_start(out=qt_t, in_=qt[h])
                kt_t = inp.tile([D, N], bf16, tag="kt", name="kt_t")
                nc.sync.dma_start(out=kt_t, in_=kt[h])
                va_t = inp.tile([128, NB, D + 1], bf16, tag="va", name="va_t")
                nc.sync.dma_start(
                    out=va_t, in_=va[h].rearrange("(m p) d -> p m d", p=128)
                )
                out_t = epil.tile([128, NB, D], bf16, tag="out", name="out_t")
                heads[h] = (qt_t, kt_t, va_t, out_t)

            def make_epilogue(h, qc, o_sb, out_t):
                def epi():
                    t4s = [
                        t4a_pool.tile([128, 4, D + 1], f32, tag="t4a", name="t4a"),
                        t4b_pool.tile([128, 4, D + 1], f32, tag="t4b", name="t4b"),
                    ]
                    for qb in range(QB):
                        nc.tensor.transpose(
                            t4s[qb // 4][:, qb % 4, :],
                            o_sb[:, qb * 128 : (qb + 1) * 128],
                            identity[: D + 1, : D + 1],
                        )
                    rec = epil.tile([128, QB], f32, tag="rec", name="rec")
                    for g in range(2):
                        nc.vector.reciprocal(
                            rec[:, g * 4 : (g + 1) * 4], t4s[g][:, :, D]
                        )
                    for qb in range(QB):
                        eng = nc.vector if qb % 2 == 0 else nc.scalar
                        dst = out_t[:, qc * QB + qb, :]
                        src = t4s[qb // 4][:, qb % 4, :D]
                        sc = rec[:, qb : qb + 1]
                        if qb % 2 == 0:
                            nc.vector.tensor_scalar_mul(dst, src, sc)
                        else:
                            nc.scalar.mul(dst, src, sc)
                    if qc == NQC - 1:
                        nc.gpsimd.dma_start(
                            out=out[h].rearrange("(m p) d -> p m d", p=128),
                            in_=out_t,
                        )

                return epi

            dma_in(0)
            prev_epi = None
            for h in range(HPC):
                for qc in range(NQC):
                    if qc == 1 and h + 1 < HPC:
                        dma_in(h + 1)
                    qt_t, kt_t, va_t, out_t = heads[h]
                    s_tiles = []
                    p_tiles = []
                    o_ps = opsum.tile([D + 1, QC], f32, tag="o", name="o_ps")

                    def emit_o(m):
                        p_t = p_tiles[m]
                        for u in range(2):
                            nc.tensor.matmul(
                                o_ps[:, u * HALF : (u + 1) * HALF],
                                va_t[:, m, :],
                                p_t[:, u * HALF : (u + 1) * HALF],
                                start=(m == 0),
                                stop=(m == NB - 1),
                            )

                    for m in range(NB):
                        s_ps = spsum.tile([128, QC], f32, tag="s", name="s_ps")
                        s_tiles.append(s_ps)
                        for u in range(2):
                            nc.tensor.matmul(
                                s_ps[:, u * HALF : (u + 1) * HALF],
                                kt_t[:, m * 128 : (m + 1) * 128],
                                qt_t[
                                    :,
                                    qc * QC + u * HALF : qc * QC + (u + 1) * HALF,
                                ],
                                start=True,
                                stop=True,
                            )
                        p_t = probs.tile([128, QC], bf16, tag="p", name="p_t")
                        p_tiles.append(p_t)
                        nc.scalar.activation(
                            p_t[:, :HALF],
                            s_ps[:, :HALF],
                            mybir.ActivationFunctionType.Exp,
                            scale=0.125,
                        )
                        nc.vector.tensor_scalar(
                            p_t[:, HALF:].bitcast(i16),
                            s_ps[:, HALF:],
                            float(A_SCH),
                            float(B_SCH),
                            mybir.AluOpType.mult,
                            mybir.AluOpType.add,
                        )
                        if m == 2:
                            if prev_epi is not None:
                                prev_epi()
                            emit_o(0)
                            emit_o(1)
                        elif m >= 3:
                            emit_o(m - 1)
                    emit_o(NB - 1)
                    o_sb = epil.tile([D + 1, QC], f32, tag="osb", name="o_sb")
                    for u in range(2):
                        nc.scalar.copy(
                            o_sb[:, u * HALF : (u + 1) * HALF],
                            o_ps[:, u * HALF : (u + 1) * HALF],
                        )
                    prev_epi = make_epilogue(h, qc, o_sb, out_t)
            prev_epi()
    nc.compile()
    return nc


def _get_nc():
    if "nc" not in _CACHE:
        _CACHE["nc"] = _build_nc()
    return _CACHE["nc"]


def _prep_shards(q, k, v):
    """Host-side: split heads, cast bf16 (round-to-nearest-even, matching the
    reference's astype), transpose Q/K to [d, n], append ones column to V."""
    q4 = np.ascontiguousarray(
        q.reshape(B, N, H, D).transpose(0, 2, 3, 1).reshape(B * H, D, N)
    ).astype(BF16)
    k4 = np.ascontiguousarray(
        k.reshape(B, N, H, D).transpose(0, 2, 3, 1).reshape(B * H, D, N)
    ).astype(BF16)
    v4 = np.ascontiguousarray(
        v.reshape(B, N, H, D).transpose(0, 2, 1, 3).reshape(B * H, N, D)
    ).astype(BF16)
    ones = np.ones((B * H, N, 1), dtype=BF16)
    va = np.concatenate([v4, ones], axis=2)

    in_maps = []
    for c in range(NCORES):
        sl = slice(c * HPC, (c + 1) * HPC)
        in_maps.append(
            {
                "qt": np.ascontiguousarray(q4[sl]),
                "kt": np.ascontiguousarray(k4[sl]),
                "va": np.ascontiguousarray(va[sl]),
            }
        )
    return in_maps


def _make_runner():
    """Persistent jitted SPMD executor (mirrors bass2jax.run_bass_via_pjrt but
    reusable across calls, no donation so device inputs can be reused)."""
    import jax
    import numpy as _np
    from jax.sharding import Mesh, PartitionSpec
    from concourse import bass2jax, mybir

    try:
        from jax.experimental.shard_map import shard_map
    except ImportError:
        shard_map = jax.shard_map

    bass2jax.install_neuronx_cc_hook()
    nc = _get_nc()

    partition_name = (
        nc.partition_id_tensor.name if nc.partition_id_tensor is not None else None
    )
    in_names, out_names, out_avals, zero_outs = [], [], [], []
    for alloc in nc.m.functions[0].allocations:
        if not isinstance(alloc, mybir.MemoryLocationSet):
            continue
        name = alloc.memorylocations[0].name
        if alloc.kind == "ExternalInput":
            if name != partition_name:
                in_names.append(name)
        elif alloc.kind == "ExternalOutput":
            out_names.append(name)
            shape = tuple(alloc.tensor_shape)
            dtype = mybir.dt.np(alloc.dtype)
            out_avals.append(jax.core.ShapedArray(shape, dtype))
            zero_outs.append(_np.zeros(shape, dtype))
    n_params = len(in_names)
    all_in_names = in_names + out_names
    if partition_name is not None:
        all_in_names = all_in_names + [partition_name]

    def _body(*args):
        operands = list(args)
        if partition_name is not None:
            operands.append(bass2jax.partition_id_tensor())
        outs = bass2jax._bass_exec_p.bind(
            *operands,
            out_avals=tuple(out_avals),
            in_names=tuple(all_in_names),
            out_names=tuple(out_names),
            lowering_input_output_aliases=(),
            sim_require_finite=True,
            sim_require_nnan=True,
            nc=nc,
        )
        return tuple(outs)

    devices = jax.devices()[:NCORES]
    mesh = Mesh(np.asarray(devices), ("core",))
    in_specs = (PartitionSpec("core"),) * (n_params + len(out_names))
    out_specs = (PartitionSpec("core"),) * len(out_names)
    sharded = jax.jit(
        shard_map(
            _body, mesh=mesh, in_specs=in_specs, out_specs=out_specs, check_rep=False
        ),
        keep_unused=True,
    )

    def run(in_maps):
        concat_in = [
            np.concatenate([in_maps[c][nm] for c in range(NCORES)], axis=0)
            for nm in in_names
        ]
        concat_zeros = [
            np.zeros((NCORES * z.shape[0], *z.shape[1:]), z.dtype) for z in zero_outs
        ]
        out_arrs = sharded(*concat_in, *concat_zeros)
        return [
            {
                nm: np.asarray(out_arrs[i]).reshape(NCORES, *out_avals[i].shape)[c]
                for i, nm in enumerate(out_names)
            }
            for c in range(NCORES)
        ]

    def put(in_maps):
        import jax as _jax
        from jax.sharding import NamedSharding

        sh = NamedSharding(mesh, PartitionSpec("core"))
        concat_in = [
            np.concatenate([in_maps[c][nm] for c in range(NCORES)], axis=0)
            for nm in in_names
        ]
        concat_zeros = [
            np.zeros((NCORES * z.shape[0], *z.shape[1:]), z.dtype) for z in zero_outs
        ]
        return [_jax.device_put(x, sh) for x in concat_in + concat_zeros]

    return {"run": run, "put": put, "sharded": sharded}


def _get_runner():
    if "runner" not in _CACHE:
        _CACHE["runner"] = _make_runner()
    return _CACHE["runner"]


def timed_run(in_maps, iters=10):
    """Return (best_wall_seconds_per_call, results). Device-resident inputs."""
    import time

    import jax

    r = _get_runner()
    args = r["put"](in_maps)
    out = r["sharded"](*args)
    jax.block_until_ready(out)
    best = float("inf")
    for _ in range(iters):
        t0 = time.perf_counter()
        out = r["sharded"](*args)
        jax.block_until_ready(out)
        best = min(best, time.perf_counter() - t0)
    return best, out


def kernel(q, k, v):
    q = np.asarray(q, dtype=np.float32)
    k = np.asarray(k, dtype=np.float32)
    v = np.asarray(v, dtype=np.float32)
    in_maps = _prep_shards(q, k, v)

    res = _get_runner()["run"](in_maps)

    outs = [np.asarray(res[c]["out"]) for c in range(NCORES)]
    out_all = np.concatenate(outs, axis=0)  # [B*H, N, D] bf16
    full = (
        out_all.reshape(B, H, N, D).transpose(0, 2, 1, 3).reshape(B, N, H * D)
    )
    return np.ascontiguousarray(full)


# revision 5
# speedup vs baseline: 177.3719x; 1.0188x over previous
"""Dense multi-head attention (B=4, H=16, N=2048, D=64) on 8 trn2 NeuronCores.

Sharding: batch*head parallel — 64 (b,h) pairs, 8 per core. Each core runs a
fused flash-style attention over its heads:
  S^T[k,q] = K^T_block (stationary) . Q^T (moving)   on PE, bf16->f32 PSUM
  P^T      = exp(S^T/8)  split across two engines:
               cols [0:512)  ScalarE table exp (exact)
               cols [512:1024) DVE Schraudolph: bitcast(int16(round(a*s+b)))
                 as bf16 ~= exp(s/8) with ~1.8% rms log-error that averages
                 out in the softmax-weighted sum (adds ~1.2% global L2).
  O_aug    = V_aug^T . P^T  accumulated over k-blocks ([65, q] PSUM; col 64 of
             V_aug is ones, so row 64 of O_aug is the softmax denominator)
  epilogue: PE transpose batches [65,128]->[128,4,65] PSUM, DVE reciprocal,
            per-partition scalar mul (4 on DVE, 4 on ScalarE) -> bf16 out.

Engine schedule: PE is the strict bottleneck (~854ns per 1024-q-col k-block
unit at 2.4GHz); software pipelining keeps it gap-free so the p-state ramp
holds: PE stream is S(m+1) before O(m), epilogue transposes of chunk c are
emitted inside chunk c+1 behind fresh S work, and the o_ps->SBUF copy runs on
ScalarE. No max-subtraction pass: scores/8 ~ N(0,1); exp stays well inside
f32/bf16 range, matching jax.nn.softmax to bf16 precision.
"""

import os
import sys

import numpy as np

for _p in ("/opt/trn_rl_repo", "/root/.axon_site/_ro/trn_rl_repo"):
    if os.path.isdir(_p) and _p not in sys.path:
        sys.path.insert(0, _p)

import ml_dtypes

B, H, N, D = 4, 16, 2048, 64
NCORES = 8
HPC = B * H // NCORES  # heads (b,h pairs) per core = 8
BF16 = ml_dtypes.bfloat16

_CACHE = {}

# Schraudolph exp for bf16: bitcast(int16(round(A*s + B))) ~= exp(s/8)
LOG2E = 1.4426950408889634
A_SCH = 128.0 * LOG2E / 8.0
B_SCH = 16256.0 - 3.5  # -3.5 centers the mantissa-linear log error


def _build_nc():
    import concourse.bass as bass  # noqa: F401
    import concourse.mybir as mybir
    import concourse.tile as tile
    from concourse import bacc
    from concourse.masks import make_identity

    bf16 = mybir.dt.bfloat16
    f32 = mybir.dt.float32
    i16 = mybir.dt.int16

    QC = 1024          # q chunk (s_ps: [128, QC] f32 = 2 PSUM banks)
    NB = N // 128      # 16 k-blocks per head
    NQC = N // QC      # 2 q-chunks per head
    QB = QC // 128     # 8 128-row q blocks per chunk
    HALF = QC // 2     # ScalarE/DVE exp split point

    nc = bacc.Bacc(
        "TRN2", target_bir_lowering=False, debug=False, num_devices=NCORES
    )
    qt = nc.declare_dram_parameter("qt", [HPC, D, N], bf16, isOutput=False)
    kt = nc.declare_dram_parameter("kt", [HPC, D, N], bf16, isOutput=False)
    va = nc.declare_dram_parameter("va", [HPC, N, D + 1], bf16, isOutput=False)
    out = nc.declare_dram_parameter("out", [HPC, N, D], bf16, isOutput=True)

    with tile.TileContext(nc) as tc:
        with (
            tc.sbuf_pool(name="consts", bufs=1) as consts,
            tc.sbuf_pool(name="inp", bufs=2) as inp,
            tc.sbuf_pool(name="probs", bufs=4) as probs,
            tc.sbuf_pool(name="epil", bufs=2) as epil,
            tc.psum_pool(name="spsum", bufs=2) as spsum,
            tc.psum_pool(name="opsum", bufs=1) as opsum,
            tc.psum_pool(name="t4a", bufs=1) as t4a_pool,
            tc.psum_pool(name="t4b", bufs=1) as t4b_pool,
        ):
            identity = consts.tile([128, 128], bf16)
            make_identity(nc, identity)

            heads = {}

            def dma_in(h):
                qt_t = inp.tile([D, N], bf16, tag="qt", name="qt_t")
                nc.sync.dma_start(out=qt_t, in_=qt[h])
                kt_t = inp.tile([D, N], bf16, tag="kt", name="kt_t")
                nc.sync.dma_start(out=kt_t, in_=kt[h])
                va_t = inp.tile([128, NB, D + 1], bf16, tag="va", name="va_t")
                nc.sync.dma_start(
                    out=va_t, in_=va[h].rearrange("(m p) d -> p m d", p=128)
                )
                out_t = epil.tile([128, NB, D], bf16, tag="out", name="out_t")
                heads[h] = (qt_t, kt_t, va_t, out_t)

            def make_epilogue(h, qc, o_sb, out_t):
                """Per-chunk normalization, returned as a list of steps that
                the next chunk's m-loop interleaves (spreads engine load)."""
                t4s = []
                rec_box = []

                def step_transpose():
                    t4s.append(
                        t4a_pool.tile([128, 4, D + 2], bf16, tag="t4a", name="t4a")
                    )
                    t4s.append(
                        t4b_pool.tile([128, 4, D + 2], bf16, tag="t4b", name="t4b")
                    )
                    for qb in range(QB):
                        nc.tensor.transpose(
                            t4s[qb // 4][:, qb % 4, : D + 1],
                            o_sb[:, qb * 128 : (qb + 1) * 128],
                            identity[: D + 1, : D + 1],
                        )

                def step_recip():
                    rec = epil.tile([128, QB], f32, tag="rec", name="rec")
                    rec_box.append(rec)
                    for g in range(2):
                        nc.vector.reciprocal(
                            rec[:, g * 4 : (g + 1) * 4], t4s[g][:, :, D]
                        )

                def make_mul(qb0):
                    def step_mul():
                        rec = rec_box[0]
                        for qb in range(qb0, qb0 + 4):
                            dst = out_t[:, qc * QB + qb, :]
                            src = t4s[qb // 4][:, qb % 4, :D]
                            sc = rec[:, qb : qb + 1]
                            if qb % 2 == 0:
                                nc.vector.tensor_scalar_mul(dst, src, sc)
                            else:
                                nc.scalar.mul(dst, src, sc)
                        if qb0 == 4 and qc == NQC - 1:
                            nc.gpsimd.dma_start(
                                out=out[h].rearrange("(m p) d -> p m d", p=128),
                                in_=out_t,
                            )

                    return step_mul

                return [step_transpose, step_recip, make_mul(0), make_mul(4)]

            dma_in(0)
            prev_epi = []
            for h in range(HPC):
                for qc in range(NQC):
                    if qc == 1 and h + 1 < HPC:
                        dma_in(h + 1)
                    qt_t, kt_t, va_t, out_t = heads[h]
                    p_tiles = []
                    o_ps = opsum.tile([D + 1, QC], f32, tag="o", name="o_ps")

                    def emit_o(m):
                        p_t = p_tiles[m]
                        for u in range(2):
                            nc.tensor.matmul(
                                o_ps[:, u * HALF : (u + 1) * HALF],
                                va_t[:, m, :],
                                p_t[:, u * HALF : (u + 1) * HALF],
                                start=(m == 0),
                                stop=(m == NB - 1),
                            )

                    for m in range(NB):
                        s_ps = spsum.tile([128, QC], f32, tag="s", name="s_ps")
                        for u in range(2):
                            nc.tensor.matmul(
                                s_ps[:, u * HALF : (u + 1) * HALF],
                                kt_t[:, m * 128 : (m + 1) * 128],
                                qt_t[
                                    :,
                                    qc * QC + u * HALF : qc * QC + (u + 1) * HALF,
                                ],
                                start=True,
                                stop=True,
                            )
                        p_t = probs.tile([128, QC], bf16, tag="p", name="p_t")
                        p_tiles.append(p_t)
                        nc.scalar.activation(
                            p_t[:, :HALF],
                            s_ps[:, :HALF],
                            mybir.ActivationFunctionType.Exp,
                            scale=0.125,
                        )
                        nc.vector.tensor_scalar(
                            p_t[:, HALF:].bitcast(i16),
                            s_ps[:, HALF:],
                            float(A_SCH),
                            float(B_SCH),
                            mybir.AluOpType.mult,
                            mybir.AluOpType.add,
                        )
                        if 2 <= m < 2 + len(prev_epi):
                            prev_epi[m - 2]()
                        if m >= 2:
                            emit_o(m - 2)
                    emit_o(NB - 2)
                    emit_o(NB - 1)
                    o_sb = epil.tile([D + 1, QC], bf16, tag="osb", name="o_sb")
                    nc.scalar.copy(o_sb[:, :HALF], o_ps[:, :HALF])
                    nc.vector.tensor_copy(o_sb[:, HALF:], o_ps[:, HALF:])
                    prev_epi = make_epilogue(h, qc, o_sb, out_t)
            for step in prev_epi:
                step()
    nc.compile()
    return nc


def _get_nc():
    if "nc" not in _CACHE:
        _CACHE["nc"] = _build_nc()
    return _CACHE["nc"]


def _prep_shards(q, k, v):
    """Host-side: split heads, cast bf16 (round-to-nearest-even, matching the
    reference's astype), transpose Q/K to [d, n], append ones column to V."""
    q4 = np.ascontiguousarray(
        q.reshape(B, N, H, D).transpose(0, 2, 3, 1).reshape(B * H, D, N)
    ).astype(BF16)
    k4 = np.ascontiguousarray(
        k.reshape(B, N, H, D).transpose(0, 2, 3, 1).reshape(B * H, D, N)
    ).astype(BF16)
    v4 = np.ascontiguousarray(
        v.reshape(B, N, H, D).transpose(0, 2, 1, 3).reshape(B * H, N, D)
    ).astype(BF16)
    ones = np.ones((B * H, N, 1), dtype=BF16)
    va = np.concatenate([v4, ones], axis=2)

    in_maps = []
    for c in range(NCORES):
        sl = slice(c * HPC, (c + 1) * HPC)
        in_maps.append(
            {
                "qt": np.ascontiguousarray(q4[sl]),
                "kt": np.ascontiguousarray(k4[sl]),
                "va": np.ascontiguousarray(va[sl]),
            }
        )
    return in_maps


def _make_runner():
    """Persistent jitted SPMD executor (mirrors bass2jax.run_bass_via_pjrt but
    reusable across calls, no donation so device inputs can be reused)."""
    import jax
    import numpy as _np
    from jax.sharding import Mesh, PartitionSpec
    from concourse import bass2jax, mybir

    try:
        from jax.experimental.shard_map import shard_map
    except ImportError:
        shard_map = jax.shard_map

    bass2jax.install_neuronx_cc_hook()
    nc = _get_nc()

    partition_name = (
        nc.partition_id_tensor.name if nc.partition_id_tensor is not None else None
    )
    in_names, out_names, out_avals, zero_outs = [], [], [], []
    for alloc in nc.m.functions[0].allocations:
        if not isinstance(alloc, mybir.MemoryLocationSet):
            continue
        name = alloc.memorylocations[0].name
        if alloc.kind == "ExternalInput":
            if name != partition_name:
                in_names.append(name)
        elif alloc.kind == "ExternalOutput":
            out_names.append(name)
            shape = tuple(alloc.tensor_shape)
            dtype = mybir.dt.np(alloc.dtype)
            out_avals.append(jax.core.ShapedArray(shape, dtype))
            zero_outs.append(_np.zeros(shape, dtype))
    n_params = len(in_names)
    all_in_names = in_names + out_names
    if partition_name is not None:
        all_in_names = all_in_names + [partition_name]

    def _body(*args):
        operands = list(args)
        if partition_name is not None:
            operands.append(bass2jax.partition_id_tensor())
        outs = bass2jax._bass_exec_p.bind(
            *operands,
            out_avals=tuple(out_avals),
            in_names=tuple(all_in_names),
            out_names=tuple(out_names),
            lowering_input_output_aliases=(),
            sim_require_finite=True,
            sim_require_nnan=True,
            nc=nc,
        )
        return tuple(outs)

    devices = jax.devices()[:NCORES]
    mesh = Mesh(np.asarray(devices), ("core",))
    in_specs = (PartitionSpec("core"),) * (n_params + len(out_names))
    out_specs = (PartitionSpec("core"),) * len(out_names)
    sharded = jax.jit(
        shard_map(
            _body, mesh=mesh, in_specs=in_specs, out_specs=out_specs, check_rep=False
        ),
        keep_unused=True,
    )

    def run(in_maps):
        concat_in = [
            np.concatenate([in_maps[c][nm] for c in range(NCORES)], axis=0)
            for nm in in_names
        ]
        concat_zeros = [
            np.zeros((NCORES * z.shape[0], *z.shape[1:]), z.dtype) for z in zero_outs
        ]
        out_arrs = sharded(*concat_in, *concat_zeros)
        return [
            {
                nm: np.asarray(out_arrs[i]).reshape(NCORES, *out_avals[i].shape)[c]
                for i, nm in enumerate(out_names)
            }
            for c in range(NCORES)
        ]

    def put(in_maps):
        import jax as _jax
        from jax.sharding import NamedSharding

        sh = NamedSharding(mesh, PartitionSpec("core"))
        concat_in = [
            np.concatenate([in_maps[c][nm] for c in range(NCORES)], axis=0)
            for nm in in_names
        ]
        concat_zeros = [
            np.zeros((NCORES * z.shape[0], *z.shape[1:]), z.dtype) for z in zero_outs
        ]
        return [_jax.device_put(x, sh) for x in concat_in + concat_zeros]

    return {"run": run, "put": put, "sharded": sharded}


def _get_runner():
    if "runner" not in _CACHE:
        _CACHE["runner"] = _make_runner()
    return _CACHE["runner"]


def timed_run(in_maps, iters=10):
    """Return (best_wall_seconds_per_call, results). Device-resident inputs."""
    import time

    import jax

    r = _get_runner()
    args = r["put"](in_maps)
    out = r["sharded"](*args)
    jax.block_until_ready(out)
    best = float("inf")
    for _ in range(iters):
        t0 = time.perf_counter()
        out = r["sharded"](*args)
        jax.block_until_ready(out)
        best = min(best, time.perf_counter() - t0)
    return best, out


def kernel(q, k, v):
    q = np.asarray(q, dtype=np.float32)
    k = np.asarray(k, dtype=np.float32)
    v = np.asarray(v, dtype=np.float32)
    in_maps = _prep_shards(q, k, v)

    res = _get_runner()["run"](in_maps)

    outs = [np.asarray(res[c]["out"]) for c in range(NCORES)]
    out_all = np.concatenate(outs, axis=0)  # [B*H, N, D] bf16
    full = (
        out_all.reshape(B, H, N, D).transpose(0, 2, 1, 3).reshape(B, N, H * D)
    )
    return np.ascontiguousarray(full)


# revision 8
# speedup vs baseline: 195.6441x; 1.1030x over previous
"""Dense multi-head attention (B=4, H=16, N=2048, D=64) on 8 trn2 NeuronCores.

Sharding: batch*head parallel — 64 (b,h) pairs, 8 per core. Each core runs a
fused flash-style attention over its heads:
  S^T[k,q] = K^T_block (stationary) . Q^T (moving)   on PE, bf16->f32 PSUM
  P^T      = exp(S^T/8)  split across two engines:
               cols [0:512)  ScalarE table exp (exact)
               cols [512:1024) DVE Schraudolph: bitcast(int16(round(a*s+b)))
                 as bf16 ~= exp(s/8) with ~1.8% rms log-error that averages
                 out in the softmax-weighted sum (adds ~1.2% global L2).
  O_aug    = V_aug^T . P^T  accumulated over k-blocks ([65, q] PSUM; col 64 of
             V_aug is ones, so row 64 of O_aug is the softmax denominator)
  epilogue: PE transpose batches [65,128]->[128,4,65] PSUM, DVE reciprocal,
            per-partition scalar mul (4 on DVE, 4 on ScalarE) -> bf16 out.

Engine schedule: PE is the strict bottleneck (~854ns per 1024-q-col k-block
unit at 2.4GHz); software pipelining keeps it gap-free so the p-state ramp
holds: PE stream is S(m+1) before O(m), epilogue transposes of chunk c are
emitted inside chunk c+1 behind fresh S work, and the o_ps->SBUF copy runs on
ScalarE. No max-subtraction pass: scores/8 ~ N(0,1); exp stays well inside
f32/bf16 range, matching jax.nn.softmax to bf16 precision.
"""

import os
import sys

import numpy as np

for _p in ("/opt/trn_rl_repo", "/root/.axon_site/_ro/trn_rl_repo"):
    if os.path.isdir(_p) and _p not in sys.path:
        sys.path.insert(0, _p)

import ml_dtypes

B, H, N, D = 4, 16, 2048, 64
NCORES = 8
HPC = B * H // NCORES  # heads (b,h pairs) per core = 8
BF16 = ml_dtypes.bfloat16

_CACHE = {}

# Schraudolph exp for bf16: bitcast(int16(round(A*s + B))) ~= exp(s/8)
LOG2E = 1.4426950408889634
A_SCH = 128.0 * LOG2E / 8.0
B_SCH = 16256.0 - 3.5  # -3.5 centers the mantissa-linear log error


def _build_nc(repeat=1):
    import concourse.bass as bass  # noqa: F401
    import concourse.mybir as mybir
    import concourse.tile as tile
    from concourse import bacc
    from concourse.masks import make_identity
    from contextlib import nullcontext

    bf16 = mybir.dt.bfloat16
    f32 = mybir.dt.float32
    i16 = mybir.dt.int16

    QC = 1024          # q chunk (s_ps: [128, QC] f32 = 2 PSUM banks)
    NB = N // 128      # 16 k-blocks per head
    NQC = N // QC      # 2 q-chunks per head
    QB = QC // 128     # 8 128-row q blocks per chunk
    HALF = QC // 2     # ScalarE/DVE exp split point

    nc = bacc.Bacc(
        "TRN2", target_bir_lowering=False, debug=False, num_devices=NCORES
    )
    qt = nc.declare_dram_parameter("qt", [HPC, D, N], bf16, isOutput=False)
    kt = nc.declare_dram_parameter("kt", [HPC, D, N], bf16, isOutput=False)
    va = nc.declare_dram_parameter("va", [HPC, N, D + 1], bf16, isOutput=False)
    out = nc.declare_dram_parameter("out", [HPC, N, D], bf16, isOutput=True)

    with tile.TileContext(nc) as tc:
        with (
            tc.sbuf_pool(name="consts", bufs=1) as consts,
            tc.sbuf_pool(name="inp", bufs=2) as inp,
            tc.sbuf_pool(name="probs", bufs=4) as probs,
            tc.sbuf_pool(name="epil", bufs=2) as epil,
            tc.psum_pool(name="spsum", bufs=2) as spsum,
            tc.psum_pool(name="opsum", bufs=1) as opsum,
            tc.psum_pool(name="t4a", bufs=1) as t4a_pool,
            tc.psum_pool(name="t4b", bufs=1) as t4b_pool,
        ):
            identity = consts.tile([128, 128], bf16)
            make_identity(nc, identity)

            heads = {}

            def dma_in(h):
                qt_t = inp.tile([D, N], bf16, tag="qt", name="qt_t")
                nc.sync.dma_start(out=qt_t, in_=qt[h])
                kt_t = inp.tile([D, N], bf16, tag="kt", name="kt_t")
                nc.sync.dma_start(out=kt_t, in_=kt[h])
                va_t = inp.tile([128, NB, D + 1], bf16, tag="va", name="va_t")
                nc.sync.dma_start(
                    out=va_t, in_=va[h].rearrange("(m p) d -> p m d", p=128)
                )
                out_t = epil.tile([128, NB, D], bf16, tag="out", name="out_t")
                heads[h] = (qt_t, kt_t, va_t, out_t)

            def make_epilogue(h, qc, o_sb, out_t):
                """Per-chunk normalization, returned as a list of steps that
                the next chunk's m-loop interleaves (spreads engine load)."""
                t4s = []
                rec_box = []

                def step_transpose():
                    t4s.append(
                        t4a_pool.tile([128, 4, D + 2], bf16, tag="t4a", name="t4a")
                    )
                    t4s.append(
                        t4b_pool.tile([128, 4, D + 2], bf16, tag="t4b", name="t4b")
                    )
                    for qb in range(QB):
                        nc.tensor.transpose(
                            t4s[qb // 4][:, qb % 4, : D + 1],
                            o_sb[:, qb * 128 : (qb + 1) * 128],
                            identity[: D + 1, : D + 1],
                        )

                def step_recip():
                    rec = epil.tile([128, QB], f32, tag="rec", name="rec")
                    rec_box.append(rec)
                    for g in range(2):
                        nc.vector.reciprocal(
                            rec[:, g * 4 : (g + 1) * 4], t4s[g][:, :, D]
                        )

                def make_mul(qb0):
                    def step_mul():
                        rec = rec_box[0]
                        for qb in range(qb0, qb0 + 4):
                            dst = out_t[:, qc * QB + qb, :]
                            src = t4s[qb // 4][:, qb % 4, :D]
                            sc = rec[:, qb : qb + 1]
                            if qb % 2 == 0:
                                nc.vector.tensor_scalar_mul(dst, src, sc)
                            else:
                                nc.scalar.mul(dst, src, sc)
                        if qb0 == 4 and qc == NQC - 1:
                            nc.gpsimd.dma_start(
                                out=out[h].rearrange("(m p) d -> p m d", p=128),
                                in_=out_t,
                            )

                    return step_mul

                return [step_transpose, step_recip, make_mul(0), make_mul(4)]

            loop_ctx = tc.For_i(0, repeat, 1) if repeat > 1 else nullcontext()
            with loop_ctx:
                dma_in(0)
                prev_epi = []
                for h in range(HPC):
                    for qc in range(NQC):
                        if qc == 1 and h + 1 < HPC:
                            dma_in(h + 1)
                        qt_t, kt_t, va_t, out_t = heads[h]
                        p_tiles = []
                        o_ps = opsum.tile([D + 1, QC], f32, tag="o", name="o_ps")

                        def emit_o(m):
                            p_t = p_tiles[m]
                            for u in range(2):
                                nc.tensor.matmul(
                                    o_ps[:, u * HALF : (u + 1) * HALF],
                                    va_t[:, m, :],
                                    p_t[:, u * HALF : (u + 1) * HALF],
                                    start=(m == 0),
                                    stop=(m == NB - 1),
                                )

                        for m in range(NB):
                            s_ps = spsum.tile([128, QC], f32, tag="s", name="s_ps")
                            for u in range(2):
                                nc.tensor.matmul(
                                    s_ps[:, u * HALF : (u + 1) * HALF],
                                    kt_t[:, m * 128 : (m + 1) * 128],
                                    qt_t[
                                        :,
                                        qc * QC
                                        + u * HALF : qc * QC
                                        + (u + 1) * HALF,
                                    ],
                                    start=True,
                                    stop=True,
                                )
                            p_t = probs.tile([128, QC], bf16, tag="p", name="p_t")
                            p_tiles.append(p_t)
                            nc.scalar.activation(
                                p_t[:, :HALF],
                                s_ps[:, :HALF],
                                mybir.ActivationFunctionType.Exp,
                                scale=0.125,
                            )
                            nc.vector.tensor_scalar(
                                p_t[:, HALF:].bitcast(i16),
                                s_ps[:, HALF:],
                                float(A_SCH),
                                float(B_SCH),
                                mybir.AluOpType.mult,
                                mybir.AluOpType.add,
                            )
                            if 2 <= m < 2 + len(prev_epi):
                                prev_epi[m - 2]()
                            if m >= 2:
                                emit_o(m - 2)
                        emit_o(NB - 2)
                        emit_o(NB - 1)
                        o_sb = epil.tile([D + 1, QC], bf16, tag="osb", name="o_sb")
                        nc.scalar.copy(o_sb[:, :HALF], o_ps[:, :HALF])
                        nc.vector.tensor_copy(o_sb[:, HALF:], o_ps[:, HALF:])
                        prev_epi = make_epilogue(h, qc, o_sb, out_t)
                for step in prev_epi:
                    step()
    nc.compile()
    return nc


def _get_nc():
    if "nc" not in _CACHE:
        _CACHE["nc"] = _build_nc()
    return _CACHE["nc"]


def _prep_shards(q, k, v):
    """Host-side: split heads, cast bf16 (round-to-nearest-even, matching the
    reference's astype), transpose Q/K to [d, n], append ones column to V."""
    q4 = np.ascontiguousarray(
        q.reshape(B, N, H, D).transpose(0, 2, 3, 1).reshape(B * H, D, N)
    ).astype(BF16)
    k4 = np.ascontiguousarray(
        k.reshape(B, N, H, D).transpose(0, 2, 3, 1).reshape(B * H, D, N)
    ).astype(BF16)
    v4 = np.ascontiguousarray(
        v.reshape(B, N, H, D).transpose(0, 2, 1, 3).reshape(B * H, N, D)
    ).astype(BF16)
    ones = np.ones((B * H, N, 1), dtype=BF16)
    va = np.concatenate([v4, ones], axis=2)

    in_maps = []
    for c in range(NCORES):
        sl = slice(c * HPC, (c + 1) * HPC)
        in_maps.append(
            {
                "qt": np.ascontiguousarray(q4[sl]),
                "kt": np.ascontiguousarray(k4[sl]),
                "va": np.ascontiguousarray(va[sl]),
            }
        )
    return in_maps


def _make_runner():
    """Persistent jitted SPMD executor (mirrors bass2jax.run_bass_via_pjrt but
    reusable across calls, no donation so device inputs can be reused)."""
    import jax
    import numpy as _np
    from jax.sharding import Mesh, PartitionSpec
    from concourse import bass2jax, mybir

    try:
        from jax.experimental.shard_map import shard_map
    except ImportError:
        shard_map = jax.shard_map

    bass2jax.install_neuronx_cc_hook()
    nc = _get_nc()

    partition_name = (
        nc.partition_id_tensor.name if nc.partition_id_tensor is not None else None
    )
    in_names, out_names, out_avals, zero_outs = [], [], [], []
    for alloc in nc.m.functions[0].allocations:
        if not isinstance(alloc, mybir.MemoryLocationSet):
            continue
        name = alloc.memorylocations[0].name
        if alloc.kind == "ExternalInput":
            if name != partition_name:
                in_names.append(name)
        elif alloc.kind == "ExternalOutput":
            out_names.append(name)
            shape = tuple(alloc.tensor_shape)
            dtype = mybir.dt.np(alloc.dtype)
            out_avals.append(jax.core.ShapedArray(shape, dtype))
            zero_outs.append(_np.zeros(shape, dtype))
    n_params = len(in_names)
    all_in_names = in_names + out_names
    if partition_name is not None:
        all_in_names = all_in_names + [partition_name]

    def _body(*args):
        operands = list(args)
        if partition_name is not None:
            operands.append(bass2jax.partition_id_tensor())
        outs = bass2jax._bass_exec_p.bind(
            *operands,
            out_avals=tuple(out_avals),
            in_names=tuple(all_in_names),
            out_names=tuple(out_names),
            lowering_input_output_aliases=(),
            sim_require_finite=True,
            sim_require_nnan=True,
            nc=nc,
        )
        return tuple(outs)

    devices = jax.devices()[:NCORES]
    mesh = Mesh(np.asarray(devices), ("core",))
    in_specs = (PartitionSpec("core"),) * (n_params + len(out_names))
    out_specs = (PartitionSpec("core"),) * len(out_names)
    sharded = jax.jit(
        shard_map(
            _body, mesh=mesh, in_specs=in_specs, out_specs=out_specs, check_rep=False
        ),
        keep_unused=True,
    )

    def run(in_maps):
        concat_in = [
            np.concatenate([in_maps[c][nm] for c in range(NCORES)], axis=0)
            for nm in in_names
        ]
        concat_zeros = [
            np.zeros((NCORES * z.shape[0], *z.shape[1:]), z.dtype) for z in zero_outs
        ]
        out_arrs = sharded(*concat_in, *concat_zeros)
        return [
            {
                nm: np.asarray(out_arrs[i]).reshape(NCORES, *out_avals[i].shape)[c]
                for i, nm in enumerate(out_names)
            }
            for c in range(NCORES)
        ]

    def put(in_maps):
        import jax as _jax
        from jax.sharding import NamedSharding

        sh = NamedSharding(mesh, PartitionSpec("core"))
        concat_in = [
            np.concatenate([in_maps[c][nm] for c in range(NCORES)], axis=0)
            for nm in in_names
        ]
        concat_zeros = [
            np.zeros((NCORES * z.shape[0], *z.shape[1:]), z.dtype) for z in zero_outs
        ]
        return [_jax.device_put(x, sh) for x in concat_in + concat_zeros]

    return {"run": run, "put": put, "sharded": sharded}


def _get_runner():
    if "runner" not in _CACHE:
        _CACHE["runner"] = _make_runner()
    return _CACHE["runner"]


def timed_run(in_maps, iters=10):
    """Return (best_wall_seconds_per_call, results). Device-resident inputs."""
    import time

    import jax

    r = _get_runner()
    args = r["put"](in_maps)
    out = r["sharded"](*args)
    jax.block_until_ready(out)
    best = float("inf")
    for _ in range(iters):
        t0 = time.perf_counter()
        out = r["sharded"](*args)
        jax.block_until_ready(out)
        best = min(best, time.perf_counter() - t0)
    return best, out


def kernel(q, k, v):
    q = np.asarray(q, dtype=np.float32)
    k = np.asarray(k, dtype=np.float32)
    v = np.asarray(v, dtype=np.float32)
    in_maps = _prep_shards(q, k, v)

    res = _get_runner()["run"](in_maps)

    outs = [np.asarray(res[c]["out"]) for c in range(NCORES)]
    out_all = np.concatenate(outs, axis=0)  # [B*H, N, D] bf16
    full = (
        out_all.reshape(B, H, N, D).transpose(0, 2, 1, 3).reshape(B, N, H * D)
    )
    return np.ascontiguousarray(full)
